# revision 5
# baseline (speedup 1.0000x reference)
"""Contrastive loss (SimCLR-style) on 8 Trainium2 NeuronCores.

Full inputs in, full output out.  Inside: each core owns a 1024-row block
of feats.  The host passes each core a rolled copy of feats so the block
is always local rows 0..1023 (making the self-mask a static diagonal and
the program identical across cores).  The device normalizes rows,
computes the 1024x8192 cosine-similarity block in bf16 on the PE,
masks the self column, and accumulates row-wise sums of exp(cos/T) via
the scalar engine's fused accumulate.  Positive-pair similarities are
computed on-device against host-gathered partner rows.  The host maps
sums -> logsumexp and reduces the mean.
"""

from contextlib import ExitStack

import numpy as np

N, D, NCORES = 8192, 128, 8
BLK = N // NCORES            # 1024 rows per core
TPB = BLK // 128             # 8 M-tiles (of 128 rows) per core
NT = N // 128                # 64 row tiles total
TEMP = 0.07
EPS = 1e-8
MASK_SUB = 30.0              # cos - 30 -> exp((cos-30)/T) == 0 in fp32
CHUNK = 512                  # matmul moving-operand columns
QCOLS = 2048                 # psum tile columns (4 banks)
NQ = N // QCOLS              # exp chunks per M-tile row

_CACHE = {}
LAST_RESULT = None


def _emit(tc, xr, pr, ident_d, eyeneg_d, s_out, pos_out, rep=0):
    import concourse.mybir as mybir

    nc = tc.nc
    f32 = mybir.dt.float32
    bf16 = mybir.dt.bfloat16
    AF = mybir.ActivationFunctionType
    AX = mybir.AxisListType.X

    with ExitStack() as ctx:
        singles = ctx.enter_context(tc.tile_pool(name=f"singles{rep}", bufs=1))
        work = ctx.enter_context(tc.tile_pool(name=f"work{rep}", bufs=3))

        xbig = singles.tile([128, NT * D], f32, tag="xbig")      # rolled X, row-major
        nfT = singles.tile([128, N], bf16, tag="nfT")            # normalized X, transposed
        nfblk = singles.tile([128, BLK], f32, tag="nfblk")       # nf rows 0..1023, row-major
        pbig = singles.tile([128, TPB * D], f32, tag="pbig")     # partner rows, row-major
        ss = singles.tile([128, NT], f32, tag="ss")
        nrm = singles.tile([128, NT], f32, tag="nrm")
        rall = singles.tile([128, NT], f32, tag="rall")
        ssp = singles.tile([128, TPB], f32, tag="ssp")
        nrmp = singles.tile([128, TPB], f32, tag="nrmp")
        rp = singles.tile([128, TPB], f32, tag="rp")
        posv = singles.tile([128, TPB], f32, tag="posv")
        sv = singles.tile([128, TPB], f32, tag="sv")
        parts = singles.tile([128, TPB * NQ], f32, tag="parts")
        ident = singles.tile([128, 128], f32, tag="ident")
        eyeneg = singles.tile([128, 128], f32, tag="eyeneg")

        # ---- loads ----
        nc.sync.dma_start(
            out=xbig[:].rearrange("p (t d) -> p t d", d=D),
            in_=xr.rearrange("(t p) d -> p t d", p=128),
        )
        nc.sync.dma_start(
            out=pbig[:].rearrange("p (t d) -> p t d", d=D),
            in_=pr.rearrange("(t p) d -> p t d", p=128),
        )
        nc.sync.dma_start(out=ident[:], in_=ident_d)
        nc.sync.dma_start(out=eyeneg[:], in_=eyeneg_d)

        # ---- phase A: row norms -> 1/max(||x||, eps) ----
        # (tensor_tensor_reduce wedges the device on this runtime; use mul+reduce)
        for t in range(NT):
            j = work.tile([128, D], f32, tag="junk")
            nc.vector.tensor_mul(j[:], xbig[:, t * D:(t + 1) * D], xbig[:, t * D:(t + 1) * D])
            nc.vector.reduce_sum(out=ss[:, t:t + 1], in_=j[:], axis=AX)
        nc.scalar.activation(nrm[:], ss[:], AF.Sqrt)
        nc.vector.tensor_scalar_max(nrm[:], nrm[:], EPS)
        nc.vector.reciprocal(rall[:], nrm[:])

        # partner norms (batch all ACT Sqrt before any Exp: one table switch)
        for t in range(TPB):
            j = work.tile([128, D], f32, tag="junk")
            nc.vector.tensor_mul(j[:], pbig[:, t * D:(t + 1) * D], pbig[:, t * D:(t + 1) * D])
            nc.vector.reduce_sum(out=ssp[:, t:t + 1], in_=j[:], axis=AX)
        nc.scalar.activation(nrmp[:], ssp[:], AF.Sqrt)
        nc.vector.tensor_scalar_max(nrmp[:], nrmp[:], EPS)
        nc.vector.reciprocal(rp[:], nrmp[:])

        # ---- phase A2: normalize + transpose into nfT (bf16) ----
        with tc.tile_pool(name=f"tpsum{rep}", bufs=2, space="PSUM") as tpsum:
            for t in range(NT):
                if t < TPB:
                    nf_ap = nfblk[:, t * D:(t + 1) * D]
                else:
                    nf_t = work.tile([128, D], f32, tag="nf")
                    nf_ap = nf_t[:]
                nc.vector.tensor_scalar_mul(nf_ap, xbig[:, t * D:(t + 1) * D], rall[:, t:t + 1])
                pt = tpsum.tile([128, 128], f32, tag="tp")
                nc.tensor.transpose(pt[:], nf_ap, ident[:])
                nc.vector.tensor_copy(nfT[:, t * D:(t + 1) * D], pt[:])

        # ---- phase B: positive-pair cosines ----
        for t in range(TPB):
            npf = work.tile([128, D], f32, tag="nf")
            nc.vector.tensor_scalar_mul(npf[:], pbig[:, t * D:(t + 1) * D], rp[:, t:t + 1])
            j = work.tile([128, D], f32, tag="junk")
            nc.vector.tensor_mul(j[:], nfblk[:, t * D:(t + 1) * D], npf[:])
            nc.vector.reduce_sum(out=posv[:, t:t + 1], in_=j[:], axis=AX)
        nc.sync.dma_start(out=pos_out, in_=posv[:])

        # ---- phase C: similarity block + exp row-sums ----
        with (
            tc.tile_pool(name=f"mpsum{rep}", bufs=2, space="PSUM") as mpsum,
            tc.tile_pool(name=f"escratch{rep}", bufs=2) as esp,
        ):
            for m in range(TPB):
                lhsT = nfT[:, m * 128:(m + 1) * 128]
                for q in range(NQ):
                    pt = mpsum.tile([128, QCOLS], f32, tag="mp")
                    for jj in range(QCOLS // CHUNK):
                        n0 = q * QCOLS + jj * CHUNK
                        nc.tensor.matmul(
                            pt[:, jj * CHUNK:(jj + 1) * CHUNK],
                            lhsT, nfT[:, n0:n0 + CHUNK], start=True, stop=True,
                        )
                    if q == 0:
                        # self column of local row m*128+p is m*128+p (rolled input)
                        nc.vector.tensor_add(
                            pt[:, m * 128:(m + 1) * 128],
                            pt[:, m * 128:(m + 1) * 128], eyeneg[:],
                        )
                    e = esp.tile([128, QCOLS], bf16, tag="e")
                    nc.scalar.activation(
                        e[:], pt[:], AF.Exp, scale=1.0 / TEMP,
                        accum_out=parts[:, m * NQ + q:m * NQ + q + 1],
                    )
        for m in range(TPB):
            nc.vector.reduce_sum(out=sv[:, m:m + 1], in_=parts[:, m * NQ:(m + 1) * NQ], axis=AX)
        nc.sync.dma_start(out=s_out, in_=sv[:])


def _build_nc(repeats=1):
    import concourse.tile as tile
    import concourse.mybir as mybir
    from concourse import bacc

    f32 = mybir.dt.float32
    nc = bacc.Bacc(
        "TRN2", target_bir_lowering=False, debug=False,
        enable_asserts=False, num_devices=NCORES,
    )
    xr_h = nc.dram_tensor("xr", [N, D], f32, kind="ExternalInput")
    pr_h = nc.dram_tensor("partner", [BLK, D], f32, kind="ExternalInput")
    id_h = nc.dram_tensor("ident", [128, 128], f32, kind="ExternalInput")
    en_h = nc.dram_tensor("eyeneg", [128, 128], f32, kind="ExternalInput")
    s_h = nc.dram_tensor("s_out", [128, TPB], f32, kind="ExternalOutput")
    p_h = nc.dram_tensor("pos_out", [128, TPB], f32, kind="ExternalOutput")

    with tile.TileContext(nc, trace_sim=False) as tc:
        for rep in range(repeats):
            _emit(tc, xr_h.ap(), pr_h.ap(), id_h.ap(), en_h.ap(),
                  s_h.ap(), p_h.ap(), rep=rep)
    nc.compile()
    return nc


def get_nc(repeats=1):
    key = ("nc", repeats)
    if key not in _CACHE:
        _CACHE[key] = _build_nc(repeats)
    return _CACHE[key]


def make_in_maps(feats, label):
    feats = np.ascontiguousarray(np.asarray(feats, dtype=np.float32))
    label = np.asarray(label)
    pos_idx = np.argmax(label, axis=1)
    partner = feats[pos_idx]
    ident = np.eye(128, dtype=np.float32)
    eyeneg = (-MASK_SUB * np.eye(128)).astype(np.float32)
    in_maps = []
    for c in range(NCORES):
        xr = np.concatenate([feats[c * BLK:], feats[:c * BLK]], axis=0)
        in_maps.append({
            "xr": np.ascontiguousarray(xr),
            "partner": np.ascontiguousarray(partner[c * BLK:(c + 1) * BLK]),
            "ident": ident,
            "eyeneg": eyeneg,
        })
    return in_maps


def finish(results):
    """Host epilogue: per-row loss values from device sums/cosines -> mean."""
    vals = []
    for c in range(NCORES):
        s = results[c]["s_out"].astype(np.float64)        # [128, TPB]
        pos = results[c]["pos_out"].astype(np.float64)    # [128, TPB]
        lse = np.log(s)
        v = lse - pos / TEMP                              # [128, TPB]
        vals.append(v.T.reshape(-1))                      # local rows 0..1023
    loss = np.concatenate(vals).mean()
    return np.array(loss, dtype=np.float32)


def kernel(feats, label, _trace=False, _repeats=1):
    global LAST_RESULT
    from concourse.bass_utils import run_bass_kernel_spmd

    nc = get_nc(_repeats)
    in_maps = make_in_maps(feats, label)
    res = run_bass_kernel_spmd(nc, in_maps, list(range(NCORES)), trace=_trace)
    LAST_RESULT = res
    return finish(res.results)
